# revision 1
# baseline (speedup 1.0000x reference)
"""Cross-attention layer on 8 Trainium2 NeuronCores (Bass/Tile SPMD).

Sharding: tensor-parallel over heads. Each core owns 4 of the 32 heads:
it projects Q^T/K^T/V for its heads (bf16 matmuls, fp32 accumulate),
runs masked softmax attention in transposed layout (scores^T so the
softmax v-reduction is a PE ones-matmul and no attn transpose is ever
needed), then an AllToAll redistributes ctx^T from head-sharded to
token-sharded so every core runs the output projection + residual +
LayerNorm for its own 256-token slice. Host concatenates the 8 slices.

Numerics: matmul inputs bf16 (error ~1e-3 of output scale, validated
against the fp32 reference), all accumulation fp32, softmax without
max-subtraction (scores ~N(0,1), exp can't overflow), mask folded into
the exp bias, 1/sqrt(hd) folded into Wq on host, bv folded into an
effective bo on host (rows of attn sum to 1), residual+LN in fp32.
"""
import sys

sys.path.insert(0, "/opt/trn_rl_repo")

import numpy as np
import ml_dtypes

import concourse.bacc as bacc
import concourse.mybir as mybir
import concourse.tile as tile
from concourse.bass_utils import run_bass_kernel_spmd

BF16 = ml_dtypes.bfloat16

NCORES = 8
P = 128            # partitions / head dim / k-tile
H = 4096
KT = H // P        # 32 k-tiles along any H contraction
NH = 32
NHL = NH // NCORES  # 4 local heads
CW = NHL * P       # 512 local c-columns
B = 2
LB = 1024          # tokens per batch
L2 = B * LB        # 2048 total tokens
TL = L2 // NCORES  # 256 tokens per core after A2A
QW = 512           # token-quarter width in phase A
NQ = L2 // QW      # 4
NVT = L2 // P      # 16 v tiles total (8 per batch)
MSK = -1e30

_CACHE = {}

F32 = mybir.dt.float32
BF = mybir.dt.bfloat16


def _build(debug=False):
    nc = bacc.Bacc("TRN2", target_bir_lowering=False, debug=False,
                   num_devices=NCORES)

    hidT_d = nc.dram_tensor("hidT", [H, L2], BF, kind="ExternalInput")
    visT_d = nc.dram_tensor("visT", [H, L2], BF, kind="ExternalInput")
    wqT_d = nc.dram_tensor("wqT", [H, CW], BF, kind="ExternalInput")
    wkT_d = nc.dram_tensor("wkT", [H, CW], BF, kind="ExternalInput")
    wvT_d = nc.dram_tensor("wvT", [H, CW], BF, kind="ExternalInput")
    woT_d = nc.dram_tensor("woT", [H, H], BF, kind="ExternalInput")
    bqT_d = nc.dram_tensor("bqT", [P, NHL], F32, kind="ExternalInput")
    bkT_d = nc.dram_tensor("bkT", [P, NHL], F32, kind="ExternalInput")
    mskb_d = nc.dram_tensor("mskb", [P, B * 8], F32, kind="ExternalInput")
    hb_d = nc.dram_tensor("hb", [TL, H], F32, kind="ExternalInput")
    g_d = nc.dram_tensor("g", [P, H], F32, kind="ExternalInput")
    bta_d = nc.dram_tensor("bta", [P, H], F32, kind="ExternalInput")
    out_d = nc.dram_tensor("out", [TL, H], F32, kind="ExternalOutput")
    if debug:
        qT_dbg = nc.dram_tensor("qT_dbg", [P, NHL * L2], BF, kind="ExternalOutput")
        kT_dbg = nc.dram_tensor("kT_dbg", [P, NHL * L2], BF, kind="ExternalOutput")
        v_dbg = nc.dram_tensor("v_dbg", [P, NVT * CW], BF, kind="ExternalOutput")
        ctxT_dbg = nc.dram_tensor("ctxT_dbg", [P, NHL * L2], BF, kind="ExternalOutput")
        octxT_dbg = nc.dram_tensor("octxT_dbg", [P, KT * TL], BF, kind="ExternalOutput")
        xpre_dbg = nc.dram_tensor("xpre_dbg", [TL, H], F32, kind="ExternalOutput")

    with tile.TileContext(nc) as tc:
        with tc.tile_pool(name="persist", bufs=1) as pers, \
             tc.tile_pool(name="dram", bufs=1, space="DRAM") as dram:

            pqkv = tc.alloc_tile_pool(name="pqkv", bufs=1)
            qT_sb = pqkv.tile([P, NHL * L2], BF)     # Q^T/sqrt(hd): [hd, (h, l)]
            kT_sb = pqkv.tile([P, NHL * L2], BF)     # K^T: [hd, (h, v)]
            v_sb = pqkv.tile([P, NVT * CW], BF)      # V: [v, (vt, c)]
            ctxT_sb = pqkv.tile([P, NHL * L2], BF)   # ctx^T normalized: [hd, (h, l)]
            bqT_sb = pers.tile([P, NHL], F32)
            bkT_sb = pers.tile([P, NHL], F32)
            mskb_sb = pers.tile([P, B * 8], F32)
            ones_bf = pers.tile([P, 1], BF)
            ones_f32 = pers.tile([1, P], F32)
            nc.sync.dma_start(out=bqT_sb[:], in_=bqT_d[:])
            nc.sync.dma_start(out=bkT_sb[:], in_=bkT_d[:])
            nc.sync.dma_start(out=mskb_sb[:], in_=mskb_d[:])
            nc.vector.memset(ones_bf[:], 1.0)
            nc.vector.memset(ones_f32[:], 1.0)

            # ---------------- Phase A: Q^T, K^T, V projections ----------------
            with tc.tile_pool(name="phaseA", bufs=2) as pa, \
                 tc.tile_pool(name="psA", bufs=6, space="PSUM") as psA:

                def load_w(dram_t, tag="wproj"):
                    w_sb = pa.tile([P, KT * CW], BF, tag=tag, name="w_sb")
                    nc.sync.dma_start(
                        out=w_sb[:].rearrange("p (kt c) -> p kt c", kt=KT),
                        in_=dram_t[:].rearrange("(kt p) c -> p kt c", p=P))
                    return w_sb

                wq_sb = load_w(wqT_d)
                wk_sb = load_w(wkT_d)
                wv_sb = load_w(wvT_d)

                def proj_qk(x_dram, w_sb, b_sb, dst_sb):
                    for q in range(NQ):
                        xT = pa.tile([P, KT * QW], BF, tag="xT")
                        nc.sync.dma_start(
                            out=xT[:].rearrange("p (kt l) -> p kt l", kt=KT),
                            in_=x_dram[:, q * QW:(q + 1) * QW]
                                .rearrange("(kt p) l -> p kt l", p=P))
                        for h in range(NHL):
                            ps = psA.tile([P, QW], F32, tag="psA")
                            for kt in range(KT):
                                nc.tensor.matmul(
                                    ps[:],
                                    w_sb[:, kt * CW + h * P: kt * CW + (h + 1) * P],
                                    xT[:, kt * QW:(kt + 1) * QW],
                                    start=(kt == 0), stop=(kt == KT - 1))
                            nc.vector.tensor_scalar_add(
                                dst_sb[:, h * L2 + q * QW: h * L2 + (q + 1) * QW],
                                ps[:], b_sb[:, h:h + 1])

                proj_qk(hidT_d, wq_sb, bqT_sb, qT_sb)
                proj_qk(visT_d, wk_sb, bkT_sb, kT_sb)

                # V in natural [v, c] layout: lhsT = visT tile, rhs = WvT
                for q in range(NQ):
                    xT = pa.tile([P, KT * QW], BF, tag="xT")
                    nc.sync.dma_start(
                        out=xT[:].rearrange("p (kt l) -> p kt l", kt=KT),
                        in_=visT_d[:, q * QW:(q + 1) * QW]
                            .rearrange("(kt p) l -> p kt l", p=P))
                    for vt in range(4):
                        g_vt = q * 4 + vt
                        ps = psA.tile([P, CW], F32, tag="psA")
                        for kt in range(KT):
                            nc.tensor.matmul(
                                ps[:],
                                xT[:, kt * QW + vt * P: kt * QW + (vt + 1) * P],
                                wv_sb[:, kt * CW:(kt + 1) * CW],
                                start=(kt == 0), stop=(kt == KT - 1))
                        nc.scalar.copy(
                            out=v_sb[:, g_vt * CW:(g_vt + 1) * CW], in_=ps[:])

            # ---------------- Phase B: attention per (batch, head) ----------------
            with tc.tile_pool(name="phaseB", bufs=2) as pb, \
                 tc.tile_pool(name="psB", bufs=2, space="PSUM") as psB:
                for b in range(B):
                    for h in range(NHL):
                        for lh in range(2):
                            qcol = h * L2 + b * LB + lh * QW
                            attnT = pb.tile([P, 8 * QW], BF, tag="attnT", bufs=3)
                            rs_ps = psB.tile([1, QW], F32, tag="rs")
                            for vb in range(8):
                                sc_ps = psB.tile([P, QW], F32, tag="sc")
                                nc.tensor.matmul(
                                    sc_ps[:],
                                    kT_sb[:, h * L2 + b * LB + vb * P:
                                          h * L2 + b * LB + (vb + 1) * P],
                                    qT_sb[:, qcol: qcol + QW],
                                    start=True, stop=True)
                                mcol = b * 8 + vb
                                nc.scalar.activation(
                                    attnT[:, vb * QW:(vb + 1) * QW], sc_ps[:],
                                    mybir.ActivationFunctionType.Exp,
                                    bias=mskb_sb[:, mcol:mcol + 1], scale=1.0)
                                nc.tensor.matmul(
                                    rs_ps[:], ones_bf[:],
                                    attnT[:, vb * QW:(vb + 1) * QW],
                                    start=(vb == 0), stop=(vb == 7))
                            rcp_sb = pb.tile([1, QW], F32, tag="rcp")
                            nc.vector.reciprocal(rcp_sb[:], rs_ps[:])
                            rcp_ps = psB.tile([P, QW], F32, tag="rcpp")
                            nc.tensor.matmul(rcp_ps[:], ones_f32[:], rcp_sb[:],
                                             start=True, stop=True)
                            rcp_rep = pb.tile([P, QW], F32, tag="rcprep")
                            nc.scalar.copy(out=rcp_rep[:], in_=rcp_ps[:])
                            ctx_ps = psB.tile([P, QW], F32, tag="ctx")
                            for vb in range(8):
                                nc.tensor.matmul(
                                    ctx_ps[:],
                                    v_sb[:, (b * 8 + vb) * CW + h * P:
                                         (b * 8 + vb) * CW + (h + 1) * P],
                                    attnT[:, vb * QW:(vb + 1) * QW],
                                    start=(vb == 0), stop=(vb == 7))
                            nc.vector.tensor_tensor(
                                out=ctxT_sb[:, qcol: qcol + QW],
                                in0=ctx_ps[:], in1=rcp_rep[:],
                                op=mybir.AluOpType.mult)

            if debug:
                nc.sync.dma_start(out=qT_dbg[:], in_=qT_sb[:])
                nc.sync.dma_start(out=kT_dbg[:], in_=kT_sb[:])
                nc.sync.dma_start(out=v_dbg[:], in_=v_sb[:])
                nc.sync.dma_start(out=ctxT_dbg[:], in_=ctxT_sb[:])

            # ---------------- Phase C: A2A, O-proj, residual + LN ----------------
            a2a_in = dram.tile([NCORES, CW, TL], BF)
            a2a_out = dram.tile([NCORES, CW, TL], BF)
            for h in range(NHL):
                nc.sync.dma_start(
                    out=a2a_in[:, h * P:(h + 1) * P, :]
                        .rearrange("j p l -> p j l"),
                    in_=ctxT_sb[:, h * L2:(h + 1) * L2]
                        .rearrange("p (j l) -> p j l", j=NCORES))
            nc.gpsimd.collective_compute(
                "AllToAll", mybir.AluOpType.bypass,
                replica_groups=[list(range(NCORES))],
                ins=[a2a_in[:]], outs=[a2a_out[:]])
            pqkv.release()

            with tc.tile_pool(name="phaseC", bufs=2) as pc, \
                 tc.tile_pool(name="psC", bufs=4, space="PSUM") as psC, \
                 tc.tile_pool(name="phaseC1", bufs=1) as pc1:
                octxT = pc1.tile([P, KT * TL], BF)  # [hd, (i, ct, l)] = full ctx^T cols
                for i in range(NCORES):
                    nc.sync.dma_start(
                        out=octxT[:, i * (NHL * TL):(i + 1) * (NHL * TL)]
                            .rearrange("p (ct l) -> p ct l", ct=NHL),
                        in_=a2a_out[i].rearrange("(ct p) l -> p ct l", p=P))

                if debug:
                    nc.sync.dma_start(out=octxT_dbg[:], in_=octxT[:])
                g_sb = pc1.tile([P, H], F32)
                nc.sync.dma_start(out=g_sb[:], in_=g_d[:])
                bta_sb = pc1.tile([P, H], F32)
                nc.sync.dma_start(out=bta_sb[:], in_=bta_d[:])
                x_sb = [pc1.tile([P, H], F32, name=f"x_sb{lt}") for lt in range(2)]

                MCW = 512
                for mc in range(H // MCW):
                    wo_sb = pc.tile([P, KT * MCW], BF, tag="wo")
                    nc.sync.dma_start(
                        out=wo_sb[:].rearrange("p (ct m) -> p ct m", ct=KT),
                        in_=woT_d[:, mc * MCW:(mc + 1) * MCW]
                            .rearrange("(ct p) m -> p ct m", p=P))
                    for lt in range(2):
                        po = psC.tile([P, MCW], F32, tag="po")
                        for g in range(KT):
                            nc.tensor.matmul(
                                po[:],
                                octxT[:, g * TL + lt * P: g * TL + (lt + 1) * P],
                                wo_sb[:, g * MCW:(g + 1) * MCW],
                                start=(g == 0), stop=(g == KT - 1))
                        nc.scalar.copy(
                            out=x_sb[lt][:, mc * MCW:(mc + 1) * MCW], in_=po[:])

                for lt in range(2):
                    if debug:
                        nc.sync.dma_start(
                            out=xpre_dbg[lt * P:(lt + 1) * P, :],
                            in_=x_sb[lt][:])
                    hb_sb = pc.tile([P, H], F32, tag="hb")
                    nc.sync.dma_start(out=hb_sb[:],
                                      in_=hb_d[lt * P:(lt + 1) * P, :])
                    x = x_sb[lt]
                    musum = pc.tile([P, 1], F32, tag="musum")
                    nc.vector.scalar_tensor_tensor(
                        out=x[:], in0=x[:], scalar=1.0, in1=hb_sb[:],
                        op0=mybir.AluOpType.mult, op1=mybir.AluOpType.add,
                        accum_out=musum[:])
                    mu = pc.tile([P, 1], F32, tag="mu")
                    nc.scalar.mul(mu[:], musum[:], 1.0 / H)
                    nc.vector.tensor_scalar(
                        out=x[:], in0=x[:], scalar1=mu[:], scalar2=None,
                        op0=mybir.AluOpType.subtract)
                    ssq = pc.tile([P, 1], F32, tag="ssq")
                    nc.scalar.activation(
                        hb_sb[:], x[:], mybir.ActivationFunctionType.Square,
                        accum_out=ssq[:])
                    eps_sb = pc.tile([P, 1], F32, tag="eps")
                    nc.vector.memset(eps_sb[:], 1e-5)
                    std = pc.tile([P, 1], F32, tag="std")
                    nc.scalar.activation(
                        std[:], ssq[:], mybir.ActivationFunctionType.Sqrt,
                        bias=eps_sb[:], scale=1.0 / H)
                    rstd = pc.tile([P, 1], F32, tag="rstd")
                    nc.vector.reciprocal(rstd[:], std[:])
                    o_sb = pc.tile([P, H], F32, tag="hb")
                    nc.vector.scalar_tensor_tensor(
                        out=o_sb[:], in0=x[:], scalar=rstd[:], in1=g_sb[:],
                        op0=mybir.AluOpType.mult, op1=mybir.AluOpType.mult)
                    nc.vector.tensor_tensor(
                        out=o_sb[:], in0=o_sb[:], in1=bta_sb[:],
                        op=mybir.AluOpType.add)
                    nc.sync.dma_start(out=out_d[lt * P:(lt + 1) * P, :],
                                      in_=o_sb[:])

    nc.compile()
    return nc


def _prep_inputs(hidden_states, vision_features, attention_mask,
                 Wq, bq, Wk, bk, Wv, bv, Wo, bo, ln_g, ln_b):
    f = np.asarray
    hs = f(hidden_states, dtype=np.float32).reshape(L2, H)
    vf = f(vision_features, dtype=np.float32).reshape(L2, H)
    am = f(attention_mask)
    Wq, bq = f(Wq, dtype=np.float32), f(bq, dtype=np.float32)
    Wk, bk = f(Wk, dtype=np.float32), f(bk, dtype=np.float32)
    Wv, bv = f(Wv, dtype=np.float32), f(bv, dtype=np.float32)
    Wo, bo = f(Wo, dtype=np.float32), f(bo, dtype=np.float32)
    ln_g, ln_b = f(ln_g, dtype=np.float32), f(ln_b, dtype=np.float32)

    s = 1.0 / np.sqrt(P)
    hidT = np.ascontiguousarray(hs.T).astype(BF16)
    visT = np.ascontiguousarray(vf.T).astype(BF16)
    woT = np.ascontiguousarray(Wo.T).astype(BF16)
    mb = np.where(am != 0, 0.0, MSK).astype(np.float32)          # (B, LB)
    mskb = np.ascontiguousarray(
        mb.reshape(B, 8, P).transpose(2, 0, 1).reshape(P, B * 8))
    bo_eff = bo + Wo @ bv
    g_rep = np.ascontiguousarray(np.broadcast_to(ln_g, (P, H)))
    b_rep = np.ascontiguousarray(np.broadcast_to(ln_b, (P, H)))

    in_maps = []
    for c in range(NCORES):
        sl = slice(c * CW, (c + 1) * CW)
        in_maps.append({
            "hidT": hidT,
            "visT": visT,
            "wqT": np.ascontiguousarray((Wq[sl] * s).T).astype(BF16),
            "wkT": np.ascontiguousarray(Wk[sl].T).astype(BF16),
            "wvT": np.ascontiguousarray(Wv[sl].T).astype(BF16),
            "woT": woT,
            "bqT": np.ascontiguousarray((bq[sl] * s).reshape(NHL, P).T),
            "bkT": np.ascontiguousarray(bk[sl].reshape(NHL, P).T),
            "mskb": mskb,
            "hb": np.ascontiguousarray(hs[c * TL:(c + 1) * TL] + bo_eff),
            "g": g_rep,
            "bta": b_rep,
        })
    return in_maps


def kernel(**inputs) -> np.ndarray:
    key = "dbg" if inputs.pop("_debug", False) else "main"
    if key not in _CACHE:
        _CACHE[key] = _build(debug=(key == "dbg"))
    nc = _CACHE[key]
    in_maps = _prep_inputs(**inputs)
    res = run_bass_kernel_spmd(nc, in_maps, list(range(NCORES)))
    out = np.concatenate([res.results[c]["out"] for c in range(NCORES)], axis=0)
    if key == "dbg":
        kernel._dbg = res.results
    return out.reshape(B, LB, H)



# revision 8
# speedup vs baseline: 1.2940x; 1.2940x over previous
"""Cross-attention layer on 8 Trainium2 NeuronCores (Bass/Tile SPMD), v2.

Sharding: tensor-parallel over heads (4 local heads/core). v2 changes vs
the bf16 baseline:
  - fp8 (e4m3) weights + activations with DoubleRow matmuls (2 k-tiles
    per pass) for the Q/K/V projections, attn@V, and the O-projection.
    Weights are prescaled x64 on host so their sigma~1 sits in e4m3's
    normal range; the 1/64 is folded into the PSUM->SBUF copy scale.
  - attention probabilities stored as e5m2 (exp output; max e^~6 fits),
    row-sum + attn@V run as DoubleRow fp8 matmuls.
  - ctx scaled x16 into e4m3 for the AllToAll payload (1/16 folded into
    the O-proj output scale along with the two x64 weight scales: /1024).
  - the AllToAll is split into two head-pair chunks: chunk 0's collective
    overlaps chunk 1's attention, and chunk 0's O-proj (half the
    contraction, accumulated in SBUF) overlaps chunk 1's collective.
  - Wo is sent host-relayouted as [chunk, rows-of-chunk, H] so each
    chunk's 16 k-tiles load with plain 3-dim APs, streamed via a 4-buf
    pool (prefetched during attention).
Q/K stay bf16 into the scores matmul (contraction=128, DoubleRow not
applicable there, and score accuracy drives softmax quality).
"""
import sys

sys.path.insert(0, "/opt/trn_rl_repo")

import numpy as np
import ml_dtypes

import concourse.bacc as bacc
import concourse.mybir as mybir
import concourse.tile as tile
from concourse.bass_utils import run_bass_kernel_spmd

BF16 = ml_dtypes.bfloat16
FP8 = ml_dtypes.float8_e4m3

NCORES = 8
P = 128            # partitions / head dim / k-tile
H = 4096
KT = H // P        # 32 k-tiles along any H contraction
NH = 32
NHL = NH // NCORES  # 4 local heads
CW = NHL * P       # 512 local c-columns
B = 2
LB = 1024          # tokens per batch
L2 = B * LB        # 2048 total tokens
TL = L2 // NCORES  # 256 tokens per core after A2A
QW = 512           # token-quarter width in phase A
NQ = L2 // QW      # 4
NVT = L2 // P      # 16 v tiles total (8 per batch)
MCW = 512          # O-proj output column chunk
MSK = -1e30
WS = 64.0          # fp8 weight prescale
CS = 16.0          # fp8 ctx prescale for the A2A payload

_CACHE = {}

F32 = mybir.dt.float32
BF = mybir.dt.bfloat16
F8 = mybir.dt.float8e4
F8A = mybir.dt.float8e5
DRM = mybir.MatmulPerfMode.DoubleRow
COPY = mybir.ActivationFunctionType.Copy


def _build(debug=False):
    nc = bacc.Bacc("TRN2", target_bir_lowering=False, debug=False,
                   num_devices=NCORES)

    hidT_d = nc.dram_tensor("hidT", [H, L2], F8, kind="ExternalInput")
    visT_d = nc.dram_tensor("visT", [H, L2], F8, kind="ExternalInput")
    wqT_d = nc.dram_tensor("wqT", [H, CW], F8, kind="ExternalInput")
    wkT_d = nc.dram_tensor("wkT", [H, CW], F8, kind="ExternalInput")
    wvT_d = nc.dram_tensor("wvT", [H, CW], F8, kind="ExternalInput")
    # Wo pre-permuted on host: [ch, (i, ct, p), m] with ct the head-in-pair
    woT_d = nc.dram_tensor("woT", [2, H // 2, H], F8, kind="ExternalInput")
    bqT_d = nc.dram_tensor("bqT", [P, NHL], F32, kind="ExternalInput")
    bkT_d = nc.dram_tensor("bkT", [P, NHL], F32, kind="ExternalInput")
    mskb_d = nc.dram_tensor("mskb", [P, B * 8], F32, kind="ExternalInput")
    hb_d = nc.dram_tensor("hb", [TL, H], F32, kind="ExternalInput")
    g_d = nc.dram_tensor("g", [P, H], F32, kind="ExternalInput")
    bta_d = nc.dram_tensor("bta", [P, H], F32, kind="ExternalInput")
    out_d = nc.dram_tensor("out", [TL, H], F32, kind="ExternalOutput")

    SQ = 1.0 / (WS * float(np.sqrt(P)))   # Q copy scale (1/sqrt(hd) folded)
    SK = 1.0 / WS
    SO = 1.0 / (WS * CS)                  # O-proj copy scale

    with tile.TileContext(nc) as tc:
        with tc.tile_pool(name="persist", bufs=1) as pers, \
             tc.tile_pool(name="dram", bufs=1, space="DRAM") as dram:

            pqkv = tc.alloc_tile_pool(name="pqkv", bufs=1)
            qT_sb = pqkv.tile([P, NHL * L2], BF, name="qT_sb")
            kT_sb = pqkv.tile([P, NHL * L2], BF, name="kT_sb")
            v_sb = pqkv.tile([P, NVT * CW], F8, name="v_sb")
            ctx_sb = [pqkv.tile([P, 2 * L2], F8, name=f"ctx_sb{c}")
                      for c in range(2)]
            bqT_sb = pers.tile([P, NHL], F32, name="bqT_sb")
            bkT_sb = pers.tile([P, NHL], F32, name="bkT_sb")
            mskb_sb = pers.tile([P, B * 8], F32, name="mskb_sb")
            ones8 = pers.tile([P, 32], F8, name="ones8")
            ones16 = pers.tile([1, P], BF, name="ones16")
            nc.sync.dma_start(out=bqT_sb[:], in_=bqT_d[:])
            nc.sync.dma_start(out=bkT_sb[:], in_=bkT_d[:])
            nc.sync.dma_start(out=mskb_sb[:], in_=mskb_d[:])
            nc.vector.memset(ones8[:], 1.0)
            nc.vector.memset(ones16[:], CS)

            # ---------------- Phase A: Q^T, K^T (bf16 out), V (fp8 out) ----
            with tc.tile_pool(name="pw", bufs=1) as pw, \
                 tc.tile_pool(name="pax", bufs=3) as pax, \
                 tc.tile_pool(name="psA", bufs=6, space="PSUM") as psA:

                def load_w(dram_t, tag):
                    w_sb = pw.tile([P, KT * CW], F8, tag=tag, name="w_" + tag)
                    nc.sync.dma_start(
                        out=w_sb[:].rearrange("p (kt c) -> p kt c", kt=KT),
                        in_=dram_t[:].rearrange("(kt p) c -> p kt c", p=P))
                    return w_sb[:].rearrange("p (kt c) -> p kt c", kt=KT)

                wq = load_w(wqT_d, "wq")
                wk = load_w(wkT_d, "wk")
                wv = load_w(wvT_d, "wv")

                def xload(x_dram, q):
                    xT = pax.tile([P, KT * QW], F8, tag="xT", name="xT")
                    nc.sync.dma_start(
                        out=xT[:].rearrange("p (kt l) -> p kt l", kt=KT),
                        in_=x_dram[:, q * QW:(q + 1) * QW]
                            .rearrange("(kt p) l -> p kt l", p=P))
                    return xT[:].rearrange("p (kt l) -> p kt l", kt=KT)

                def proj_qk(xv, w, b_sb, dst, q, scale):
                    for h in range(NHL):
                        ps = psA.tile([P, QW], F32, tag="psA", name="psA")
                        for t in range(KT // 2):
                            nc.tensor.matmul(
                                ps[:], w[:, 2 * t:2 * t + 2, h * P:(h + 1) * P],
                                xv[:, 2 * t:2 * t + 2, :],
                                start=(t == 0), stop=(t == KT // 2 - 1),
                                perf_mode=DRM)
                        nc.vector.tensor_scalar(
                            out=dst[:, h * L2 + q * QW: h * L2 + (q + 1) * QW],
                            in0=ps[:], scalar1=scale, scalar2=b_sb[:, h:h + 1],
                            op0=mybir.AluOpType.mult,
                            op1=mybir.AluOpType.add)

                for q in range(NQ):
                    xv = xload(hidT_d, q)
                    proj_qk(xv, wq, bqT_sb, qT_sb, q, SQ)
                for q in range(NQ):
                    xv = xload(visT_d, q)
                    proj_qk(xv, wk, bkT_sb, kT_sb, q, SK)
                    for vt in range(4):
                        g_vt = q * 4 + vt
                        ps = psA.tile([P, CW], F32, tag="psA", name="psA")
                        for t in range(KT // 2):
                            nc.tensor.matmul(
                                ps[:], xv[:, 2 * t:2 * t + 2, vt * P:(vt + 1) * P],
                                wv[:, 2 * t:2 * t + 2, :],
                                start=(t == 0), stop=(t == KT // 2 - 1),
                                perf_mode=DRM)
                        nc.scalar.activation(
                            v_sb[:, g_vt * CW:(g_vt + 1) * CW], ps[:],
                            COPY, scale=SK)

            # ---------------- Phase B + chunked A2A ----------------
            a2a_in = [dram.tile([NCORES, 2 * P, TL], F8, name=f"a2ai{c}")
                      for c in range(2)]
            a2a_out = [dram.tile([NCORES, 2 * P, TL], F8, name=f"a2ao{c}")
                       for c in range(2)]
            vview = v_sb[:].rearrange("p (vt c) -> p vt c", vt=NVT)
            # two weight "rows" 16 B apart to satisfy s3_lw_dual_fp8_restrictions
            onesv = ones8[:].rearrange("p (a o) -> p a o", a=2)[:, :, 0:1]
            with tc.tile_pool(name="phaseB", bufs=2) as pb, \
                 tc.tile_pool(name="psB", bufs=2, space="PSUM") as psB:
                for ch in range(2):
                    for hh in range(2):
                        h = 2 * ch + hh
                        for b in range(B):
                            for lh in range(2):
                                qcol = h * L2 + b * LB + lh * QW
                                ccol = hh * L2 + b * LB + lh * QW
                                attnT = pb.tile([P, 8 * QW], F8A, tag="attnT",
                                                bufs=3, name="attnT")
                                av = attnT[:].rearrange("p (vb l) -> p vb l",
                                                        vb=8)
                                rs_ps = psB.tile([1, QW], F32, tag="rs",
                                                 name="rs")
                                for vb in range(8):
                                    sc_ps = psB.tile([P, QW], F32, tag="sc",
                                                     name="sc")
                                    nc.tensor.matmul(
                                        sc_ps[:],
                                        kT_sb[:, h * L2 + b * LB + vb * P:
                                              h * L2 + b * LB + (vb + 1) * P],
                                        qT_sb[:, qcol: qcol + QW],
                                        start=True, stop=True)
                                    mcol = b * 8 + vb
                                    nc.scalar.activation(
                                        av[:, vb, :], sc_ps[:],
                                        mybir.ActivationFunctionType.Exp,
                                        bias=mskb_sb[:, mcol:mcol + 1],
                                        scale=1.0)
                                for u in range(4):
                                    nc.tensor.matmul(
                                        rs_ps[:], onesv,
                                        av[:, 2 * u:2 * u + 2, :],
                                        start=(u == 0), stop=(u == 3),
                                        perf_mode=DRM)
                                rcp_sb = pb.tile([1, QW], BF, tag="rcp",
                                                 name="rcp")
                                with nc.allow_low_precision(
                                        reason="bf16 1/rowsum: 0.4% on a "
                                               "normalizer, below fp8 noise"):
                                    nc.vector.reciprocal(rcp_sb[:], rs_ps[:])
                                rcp_ps = psB.tile([P, QW], F32, tag="rcpp",
                                                  name="rcpp")
                                nc.tensor.matmul(rcp_ps[:], ones16[:],
                                                 rcp_sb[:],
                                                 start=True, stop=True)
                                rcp_rep = pb.tile([P, QW], F32, tag="rcprep",
                                                  name="rcprep")
                                nc.vector.tensor_copy(out=rcp_rep[:],
                                                      in_=rcp_ps[:])
                                ctx_ps = psB.tile([P, QW], F32, tag="ctx",
                                                  name="ctx")
                                for u in range(4):
                                    vb0 = b * 8 + 2 * u
                                    nc.tensor.matmul(
                                        ctx_ps[:],
                                        vview[:, vb0:vb0 + 2,
                                              h * P:(h + 1) * P],
                                        av[:, 2 * u:2 * u + 2, :],
                                        start=(u == 0), stop=(u == 3),
                                        perf_mode=DRM)
                                nc.vector.tensor_tensor(
                                    out=ctx_sb[ch][:, ccol: ccol + QW],
                                    in0=ctx_ps[:], in1=rcp_rep[:],
                                    op=mybir.AluOpType.mult)
                    for hh in range(2):
                        nc.sync.dma_start(
                            out=a2a_in[ch][:, hh * P:(hh + 1) * P, :]
                                .rearrange("j p l -> p j l"),
                            in_=ctx_sb[ch][:, hh * L2:(hh + 1) * L2]
                                .rearrange("p (j l) -> p j l", j=NCORES))
                    nc.gpsimd.collective_compute(
                        "AllToAll", mybir.AluOpType.bypass,
                        replica_groups=[list(range(NCORES))],
                        ins=[a2a_in[ch][:]], outs=[a2a_out[ch][:]])

            pqkv.release()

            # ---------------- Phase C: chunked O-proj + residual + LN ------
            with tc.tile_pool(name="pwo", bufs=4) as pwo, \
                 tc.tile_pool(name="psC", bufs=4, space="PSUM") as psC, \
                 tc.tile_pool(name="pc1", bufs=1) as pc1, \
                 tc.tile_pool(name="pcs", bufs=2) as pcs:
                g_sb = pc1.tile([P, H], F32, name="g_sb")
                nc.sync.dma_start(out=g_sb[:], in_=g_d[:])
                bta_sb = pc1.tile([P, H], F32, name="bta_sb")
                nc.sync.dma_start(out=bta_sb[:], in_=bta_d[:])
                x_sb = [pc1.tile([P, H], F32, name=f"x_sb{lt}")
                        for lt in range(2)]
                octx = [pc1.tile([P, 16 * TL], F8, name=f"octx{c}")
                        for c in range(2)]

                for ch in range(2):
                    for i in range(NCORES):
                        nc.sync.dma_start(
                            out=octx[ch][:, i * 2 * TL:(i + 1) * 2 * TL]
                                .rearrange("p (ct l) -> p ct l", ct=2),
                            in_=a2a_out[ch][i]
                                .rearrange("(ct p) l -> p ct l", p=P))
                    oview = octx[ch][:].rearrange("p (g l) -> p g l", g=16)
                    for mc in range(H // MCW):
                        wo_sb = pwo.tile([P, 16 * MCW], F8, tag="wo",
                                         name="wo_sb")
                        nc.sync.dma_start(
                            out=wo_sb[:].rearrange("p (g m) -> p g m", g=16),
                            in_=woT_d[ch][:, mc * MCW:(mc + 1) * MCW]
                                .rearrange("(g p) m -> p g m", p=P))
                        wview = wo_sb[:].rearrange("p (g m) -> p g m", g=16)
                        for lt in range(2):
                            po = psC.tile([P, MCW], F32, tag="po", name="po")
                            for u in range(8):
                                nc.tensor.matmul(
                                    po[:],
                                    oview[:, 2 * u:2 * u + 2,
                                          lt * P:(lt + 1) * P],
                                    wview[:, 2 * u:2 * u + 2, :],
                                    start=(u == 0), stop=(u == 7),
                                    perf_mode=DRM)
                            dst = x_sb[lt][:, mc * MCW:(mc + 1) * MCW]
                            if ch == 0:
                                nc.scalar.activation(dst, po[:], COPY,
                                                     scale=SO)
                            else:
                                nc.vector.scalar_tensor_tensor(
                                    out=dst, in0=po[:], scalar=SO, in1=dst,
                                    op0=mybir.AluOpType.mult,
                                    op1=mybir.AluOpType.add)

                for lt in range(2):
                    hb_sb = pcs.tile([P, H], F32, tag="hb", name="hb_sb")
                    nc.sync.dma_start(out=hb_sb[:],
                                      in_=hb_d[lt * P:(lt + 1) * P, :])
                    x = x_sb[lt]
                    musum = pcs.tile([P, 1], F32, tag="musum", name="musum")
                    nc.vector.scalar_tensor_tensor(
                        out=x[:], in0=x[:], scalar=1.0, in1=hb_sb[:],
                        op0=mybir.AluOpType.mult, op1=mybir.AluOpType.add,
                        accum_out=musum[:])
                    mu = pcs.tile([P, 1], F32, tag="mu", name="mu")
                    nc.scalar.mul(mu[:], musum[:], 1.0 / H)
                    nc.vector.tensor_scalar(
                        out=x[:], in0=x[:], scalar1=mu[:], scalar2=None,
                        op0=mybir.AluOpType.subtract)
                    ssq = pcs.tile([P, 1], F32, tag="ssq", name="ssq")
                    nc.scalar.activation(
                        hb_sb[:], x[:], mybir.ActivationFunctionType.Square,
                        accum_out=ssq[:])
                    eps_sb = pcs.tile([P, 1], F32, tag="eps", name="eps")
                    nc.vector.memset(eps_sb[:], 1e-5)
                    std = pcs.tile([P, 1], F32, tag="std", name="std")
                    nc.scalar.activation(
                        std[:], ssq[:], mybir.ActivationFunctionType.Sqrt,
                        bias=eps_sb[:], scale=1.0 / H)
                    rstd = pcs.tile([P, 1], F32, tag="rstd", name="rstd")
                    nc.vector.reciprocal(rstd[:], std[:])
                    o_sb = pcs.tile([P, H], F32, tag="hb", name="o_sb")
                    nc.vector.scalar_tensor_tensor(
                        out=o_sb[:], in0=x[:], scalar=rstd[:], in1=g_sb[:],
                        op0=mybir.AluOpType.mult, op1=mybir.AluOpType.mult)
                    nc.vector.tensor_tensor(
                        out=o_sb[:], in0=o_sb[:], in1=bta_sb[:],
                        op=mybir.AluOpType.add)
                    nc.sync.dma_start(out=out_d[lt * P:(lt + 1) * P, :],
                                      in_=o_sb[:])

    nc.compile()
    return nc


def _prep_inputs(hidden_states, vision_features, attention_mask,
                 Wq, bq, Wk, bk, Wv, bv, Wo, bo, ln_g, ln_b):
    f = np.asarray
    hs = f(hidden_states, dtype=np.float32).reshape(L2, H)
    vf = f(vision_features, dtype=np.float32).reshape(L2, H)
    am = f(attention_mask)
    Wq, bq = f(Wq, dtype=np.float32), f(bq, dtype=np.float32)
    Wk, bk = f(Wk, dtype=np.float32), f(bk, dtype=np.float32)
    Wv, bv = f(Wv, dtype=np.float32), f(bv, dtype=np.float32)
    Wo, bo = f(Wo, dtype=np.float32), f(bo, dtype=np.float32)
    ln_g, ln_b = f(ln_g, dtype=np.float32), f(ln_b, dtype=np.float32)

    rs = 1.0 / np.sqrt(P)
    hidT = np.ascontiguousarray(hs.T).astype(FP8)
    visT = np.ascontiguousarray(vf.T).astype(FP8)
    # Wo host relayout: [ch, (i, ct, p), m] where source row = full ctx col
    # (i*4 + ch*2 + ct)*P + p, m = out col.  woT_full = Wo.T * WS.
    woT_full = (Wo.T * WS).astype(FP8)           # [H(ctx col), H(out col)]
    w5 = woT_full.reshape(NCORES, 2, 2, P, H)    # [i, ch, ct, p, m]
    woT = np.ascontiguousarray(
        w5.transpose(1, 0, 2, 3, 4).reshape(2, H // 2, H))
    mb = np.where(am != 0, 0.0, MSK).astype(np.float32)          # (B, LB)
    mskb = np.ascontiguousarray(
        mb.reshape(B, 8, P).transpose(2, 0, 1).reshape(P, B * 8))
    bo_eff = bo + Wo @ bv
    g_rep = np.ascontiguousarray(np.broadcast_to(ln_g, (P, H)))
    b_rep = np.ascontiguousarray(np.broadcast_to(ln_b, (P, H)))

    in_maps = []
    for c in range(NCORES):
        sl = slice(c * CW, (c + 1) * CW)
        in_maps.append({
            "hidT": hidT,
            "visT": visT,
            "wqT": np.ascontiguousarray((Wq[sl] * WS).T).astype(FP8),
            "wkT": np.ascontiguousarray((Wk[sl] * WS).T).astype(FP8),
            "wvT": np.ascontiguousarray((Wv[sl] * WS).T).astype(FP8),
            "woT": woT,
            "bqT": np.ascontiguousarray((bq[sl] * rs).reshape(NHL, P).T),
            "bkT": np.ascontiguousarray(bk[sl].reshape(NHL, P).T),
            "mskb": mskb,
            "hb": np.ascontiguousarray(hs[c * TL:(c + 1) * TL] + bo_eff),
            "g": g_rep,
            "bta": b_rep,
        })
    return in_maps


def kernel(**inputs) -> np.ndarray:
    inputs.pop("_debug", False)
    if "main" not in _CACHE:
        _CACHE["main"] = _build(debug=False)
    nc = _CACHE["main"]
    in_maps = _prep_inputs(**inputs)
    res = run_bass_kernel_spmd(nc, in_maps, list(range(NCORES)))
    out = np.concatenate([res.results[c]["out"] for c in range(NCORES)], axis=0)
    return out.reshape(B, LB, H)


# revision 12
# speedup vs baseline: 1.8331x; 1.4166x over previous
"""Cross-attention layer on 8 Trainium2 NeuronCores (Bass/Tile SPMD), v2.

Sharding: tensor-parallel over heads (4 local heads/core). v2 changes vs
the bf16 baseline:
  - fp8 (e4m3) weights + activations with DoubleRow matmuls (2 k-tiles
    per pass) for the Q/K/V projections, attn@V, and the O-projection.
    Weights are prescaled x64 on host so their sigma~1 sits in e4m3's
    normal range; the 1/64 is folded into the PSUM->SBUF copy scale.
  - attention probabilities stored as e5m2 (exp output; max e^~6 fits),
    row-sum + attn@V run as DoubleRow fp8 matmuls.
  - ctx scaled x16 into e4m3 for the AllToAll payload (1/16 folded into
    the O-proj output scale along with the two x64 weight scales: /1024).
  - the AllToAll is split into two head-pair chunks: chunk 0's collective
    overlaps chunk 1's attention, and chunk 0's O-proj (half the
    contraction, accumulated in SBUF) overlaps chunk 1's collective.
  - Wo is sent host-relayouted as [chunk, rows-of-chunk, H] so each
    chunk's 16 k-tiles load with plain 3-dim APs, streamed via a 4-buf
    pool (prefetched during attention).
Q/K stay bf16 into the scores matmul (contraction=128, DoubleRow not
applicable there, and score accuracy drives softmax quality).
"""
import sys

sys.path.insert(0, "/opt/trn_rl_repo")

import numpy as np
import ml_dtypes

import concourse.bacc as bacc
import concourse.mybir as mybir
import concourse.tile as tile
from concourse.bass_utils import run_bass_kernel_spmd

BF16 = ml_dtypes.bfloat16
FP8 = ml_dtypes.float8_e4m3

NCORES = 8
P = 128            # partitions / head dim / k-tile
H = 4096
KT = H // P        # 32 k-tiles along any H contraction
NH = 32
NHL = NH // NCORES  # 4 local heads
CW = NHL * P       # 512 local c-columns
B = 2
LB = 1024          # tokens per batch
L2 = B * LB        # 2048 total tokens
TL = L2 // NCORES  # 256 tokens per core after A2A
QW = 512           # token-quarter width in phase A
NQ = L2 // QW      # 4
NVT = L2 // P      # 16 v tiles total (8 per batch)
MCW = 512          # O-proj output column chunk
MSK = -1e30
WS = 64.0          # fp8 weight prescale
CS = 16.0          # fp8 ctx prescale for the A2A payload

_CACHE = {}

F32 = mybir.dt.float32
BF = mybir.dt.bfloat16
F8 = mybir.dt.float8e4
F8A = mybir.dt.float8e5
DRM = mybir.MatmulPerfMode.DoubleRow
COPY = mybir.ActivationFunctionType.Copy


def _build(debug=False):
    nc = bacc.Bacc("TRN2", target_bir_lowering=False, debug=False,
                   num_devices=NCORES)

    hidT_d = nc.dram_tensor("hidT", [H, L2], F8, kind="ExternalInput")
    visT_d = nc.dram_tensor("visT", [H, L2], F8, kind="ExternalInput")
    wqT_d = nc.dram_tensor("wqT", [H, CW], F8, kind="ExternalInput")
    wkT_d = nc.dram_tensor("wkT", [H, CW], F8, kind="ExternalInput")
    wvT_d = nc.dram_tensor("wvT", [H, CW], F8, kind="ExternalInput")
    # Wo pre-permuted on host: [ch, (i, ct, p), m] with ct the head-in-pair
    woT_d = nc.dram_tensor("woT", [2, H // 2, H], F8, kind="ExternalInput")
    bqT_d = nc.dram_tensor("bqT", [P, NHL], F32, kind="ExternalInput")
    bkT_d = nc.dram_tensor("bkT", [P, NHL], F32, kind="ExternalInput")
    mskb_d = nc.dram_tensor("mskb", [P, B * 8], F32, kind="ExternalInput")
    hb_d = nc.dram_tensor("hb", [TL, H], F32, kind="ExternalInput")
    g_d = nc.dram_tensor("g", [P, H], F32, kind="ExternalInput")
    bta_d = nc.dram_tensor("bta", [P, H], F32, kind="ExternalInput")
    out_d = nc.dram_tensor("out", [TL, H], F32, kind="ExternalOutput")

    SQ = 1.0 / (WS * float(np.sqrt(P)))   # Q copy scale (1/sqrt(hd) folded)
    SK = 1.0 / WS
    SO = 1.0 / (WS * CS)                  # O-proj copy scale

    with tile.TileContext(nc) as tc:
        with tc.tile_pool(name="persist", bufs=1) as pers, \
             tc.tile_pool(name="dram", bufs=1, space="DRAM") as dram:

            pqkv = tc.alloc_tile_pool(name="pqkv", bufs=1)
            qT_sb = pqkv.tile([P, NHL * L2], BF, name="qT_sb")
            kT_sb = pqkv.tile([P, NHL * L2], BF, name="kT_sb")
            v_sb = pqkv.tile([P, NVT * CW], F8, name="v_sb")
            ctx_sb = [pqkv.tile([P, 2 * L2], F8, name=f"ctx_sb{c}")
                      for c in range(2)]
            bqT_sb = pers.tile([P, NHL], F32, name="bqT_sb")
            bkT_sb = pers.tile([P, NHL], F32, name="bkT_sb")
            mskb_sb = pers.tile([P, B * 8], F32, name="mskb_sb")
            ones8 = pers.tile([P, 32], F8, name="ones8")
            ones16 = pers.tile([1, P], BF, name="ones16")
            nc.sync.dma_start(out=bqT_sb[:], in_=bqT_d[:])
            nc.sync.dma_start(out=bkT_sb[:], in_=bkT_d[:])
            nc.sync.dma_start(out=mskb_sb[:], in_=mskb_d[:])
            nc.vector.memset(ones8[:], 1.0)
            nc.vector.memset(ones16[:], CS)

            a2a_in = [dram.tile([NCORES, 2 * P, TL], F8, name=f"a2ai{c}")
                      for c in range(2)]
            a2a_out = [dram.tile([NCORES, 2 * P, TL], F8, name=f"a2ao{c}")
                       for c in range(2)]
            vview = v_sb[:].rearrange("p (vt c) -> p vt c", vt=NVT)
            # two weight "rows" 16 B apart to satisfy s3_lw_dual_fp8_restrictions
            onesv = ones8[:].rearrange("p (a o) -> p a o", a=2)[:, :, 0:1]

            # ---- Phases A+B interleaved: batch-0 attention only needs ----
            # ---- vision quarters 0-1, so its exps (ACT) overlap the    ----
            # ---- quarter 2-3 projections (PE).  PSUM: psA 2 + psB 6.   ----
            with tc.tile_pool(name="pw", bufs=1) as pw, \
                 tc.tile_pool(name="pax", bufs=3) as pax, \
                 tc.tile_pool(name="psA", bufs=2, space="PSUM") as psA, \
                 tc.tile_pool(name="phaseB", bufs=2) as pb, \
                 tc.tile_pool(name="psB", bufs=2, space="PSUM") as psB:

                def load_w(dram_t, tag):
                    w_sb = pw.tile([P, KT * CW], F8, tag=tag, name="w_" + tag)
                    nc.sync.dma_start(
                        out=w_sb[:].rearrange("p (kt c) -> p kt c", kt=KT),
                        in_=dram_t[:].rearrange("(kt p) c -> p kt c", p=P))
                    return w_sb[:].rearrange("p (kt c) -> p kt c", kt=KT)

                wk = wv = None  # loaded mid-Q-proj; bound before kv() runs

                def xload(x_dram, q):
                    xT = pax.tile([P, KT * QW], F8, tag="xT", name="xT")
                    nc.sync.dma_start(
                        out=xT[:].rearrange("p (kt l) -> p kt l", kt=KT),
                        in_=x_dram[:, q * QW:(q + 1) * QW]
                            .rearrange("(kt p) l -> p kt l", p=P))
                    return xT[:].rearrange("p (kt l) -> p kt l", kt=KT)

                def proj_qk(xv, w, b_sb, dst, q, scale):
                    for h in range(NHL):
                        ps = psA.tile([P, QW], F32, tag="psA", name="psA")
                        for t in range(KT // 2):
                            nc.tensor.matmul(
                                ps[:], w[:, 2 * t:2 * t + 2, h * P:(h + 1) * P],
                                xv[:, 2 * t:2 * t + 2, :],
                                start=(t == 0), stop=(t == KT // 2 - 1),
                                perf_mode=DRM)
                        nc.vector.tensor_scalar(
                            out=dst[:, h * L2 + q * QW: h * L2 + (q + 1) * QW],
                            in0=ps[:], scalar1=scale, scalar2=b_sb[:, h:h + 1],
                            op0=mybir.AluOpType.mult,
                            op1=mybir.AluOpType.add)

                def kv(q):
                    xv = xload(visT_d, q)
                    proj_qk(xv, wk, bkT_sb, kT_sb, q, SK)
                    for vt in range(4):
                        g_vt = q * 4 + vt
                        ps = psA.tile([P, CW], F32, tag="psA", name="psA")
                        for t in range(KT // 2):
                            nc.tensor.matmul(
                                ps[:], xv[:, 2 * t:2 * t + 2, vt * P:(vt + 1) * P],
                                wv[:, 2 * t:2 * t + 2, :],
                                start=(t == 0), stop=(t == KT // 2 - 1),
                                perf_mode=DRM)
                        # DVE (not ACT) so phase-B exps never block the
                        # psA slot recycle feeding the PE
                        nc.vector.tensor_scalar(
                            out=v_sb[:, g_vt * CW:(g_vt + 1) * CW],
                            in0=ps[:], scalar1=SK, scalar2=None,
                            op0=mybir.AluOpType.mult)

                def attn(h, b, lh):
                    qcol = h * L2 + b * LB + lh * QW
                    ccol = (h % 2) * L2 + b * LB + lh * QW
                    attnT = pb.tile([P, 8 * QW], F8A, tag="attnT",
                                    bufs=3, name="attnT")
                    av = attnT[:].rearrange("p (vb l) -> p vb l", vb=8)
                    rs_ps = psB.tile([1, QW], F32, tag="rs", bufs=1,
                                     name="rs")
                    for vb in range(8):
                        sc_ps = psB.tile([P, QW], F32, tag="sc", name="sc")
                        nc.tensor.matmul(
                            sc_ps[:],
                            kT_sb[:, h * L2 + b * LB + vb * P:
                                  h * L2 + b * LB + (vb + 1) * P],
                            qT_sb[:, qcol: qcol + QW],
                            start=True, stop=True)
                        nc.scalar.activation(
                            av[:, vb, :], sc_ps[:],
                            mybir.ActivationFunctionType.Exp,
                            bias=mskb_sb[:, b * 8 + vb: b * 8 + vb + 1],
                            scale=1.0)
                    for u in range(4):
                        nc.tensor.matmul(
                            rs_ps[:], onesv, av[:, 2 * u:2 * u + 2, :],
                            start=(u == 0), stop=(u == 3), perf_mode=DRM)
                    rcp_sb = pb.tile([1, QW], BF, tag="rcp", name="rcp")
                    with nc.allow_low_precision(
                            reason="bf16 1/rowsum: 0.4% on a normalizer, "
                                   "below fp8 noise"):
                        nc.vector.reciprocal(rcp_sb[:], rs_ps[:])
                    rcp_ps = psB.tile([P, QW], F32, tag="rcpp", bufs=1,
                                      name="rcpp")
                    nc.tensor.matmul(rcp_ps[:], ones16[:], rcp_sb[:],
                                     start=True, stop=True)
                    rcp_rep = pb.tile([P, QW], F32, tag="rcprep",
                                      name="rcprep")
                    nc.vector.tensor_copy(out=rcp_rep[:], in_=rcp_ps[:])
                    ctx_ps = psB.tile([P, QW], F32, tag="ctx", name="ctx")
                    for u in range(4):
                        vb0 = b * 8 + 2 * u
                        nc.tensor.matmul(
                            ctx_ps[:],
                            vview[:, vb0:vb0 + 2, h * P:(h + 1) * P],
                            av[:, 2 * u:2 * u + 2, :],
                            start=(u == 0), stop=(u == 3), perf_mode=DRM)
                    nc.vector.tensor_tensor(
                        out=ctx_sb[h // 2][:, ccol: ccol + QW],
                        in0=ctx_ps[:], in1=rcp_rep[:],
                        op=mybir.AluOpType.mult)

                def a2a(ch):
                    for hh in range(2):
                        nc.sync.dma_start(
                            out=a2a_in[ch][:, hh * P:(hh + 1) * P, :]
                                .rearrange("j p l -> p j l"),
                            in_=ctx_sb[ch][:, hh * L2:(hh + 1) * L2]
                                .rearrange("p (j l) -> p j l", j=NCORES))
                    nc.gpsimd.collective_compute(
                        "AllToAll", mybir.AluOpType.bypass,
                        replica_groups=[list(range(NCORES))],
                        ins=[a2a_in[ch][:]], outs=[a2a_out[ch][:]])

                # wq + first x quarter hit the DMA queues first so the PE
                # starts ~20us earlier; wk/wv stream in behind them
                wq = load_w(wqT_d, "wq")
                for q in range(NQ):
                    xv = xload(hidT_d, q)
                    if q == 1:
                        wk = load_w(wkT_d, "wk")
                    elif q == 2:
                        wv = load_w(wvT_d, "wv")
                    proj_qk(xv, wq, bqT_sb, qT_sb, q, SQ)
                kv(0)
                kv(1)
                # batch-0 attention: needs only vision quarters 0-1; its
                # ACT exps overlap the quarter 2-3 projections on PE
                for h in range(NHL):
                    for lh in range(2):
                        attn(h, 0, lh)
                kv(2)
                kv(3)
                for h in (0, 1):
                    for lh in range(2):
                        attn(h, 1, lh)
                a2a(0)
                for h in (2, 3):
                    for lh in range(2):
                        attn(h, 1, lh)
                a2a(1)

            pqkv.release()

            # ---------------- Phase C: chunked O-proj + residual + LN ------
            with tc.tile_pool(name="pwo", bufs=4) as pwo, \
                 tc.tile_pool(name="psC", bufs=4, space="PSUM") as psC, \
                 tc.tile_pool(name="pc1", bufs=1) as pc1, \
                 tc.tile_pool(name="pcs", bufs=2) as pcs:
                g_sb = pc1.tile([P, H], F32, name="g_sb")
                nc.sync.dma_start(out=g_sb[:], in_=g_d[:])
                bta_sb = pc1.tile([P, H], F32, name="bta_sb")
                nc.sync.dma_start(out=bta_sb[:], in_=bta_d[:])
                x_sb = [pc1.tile([P, H], F32, name=f"x_sb{lt}")
                        for lt in range(2)]
                octx = [pc1.tile([P, 16 * TL], F8, name=f"octx{c}")
                        for c in range(2)]

                for ch in range(2):
                    for i in range(NCORES):
                        nc.sync.dma_start(
                            out=octx[ch][:, i * 2 * TL:(i + 1) * 2 * TL]
                                .rearrange("p (ct l) -> p ct l", ct=2),
                            in_=a2a_out[ch][i]
                                .rearrange("(ct p) l -> p ct l", p=P))
                    oview = octx[ch][:].rearrange("p (g l) -> p g l", g=16)
                    for mc in range(H // MCW):
                        wo_sb = pwo.tile([P, 16 * MCW], F8, tag="wo",
                                         name="wo_sb")
                        nc.sync.dma_start(
                            out=wo_sb[:].rearrange("p (g m) -> p g m", g=16),
                            in_=woT_d[ch][:, mc * MCW:(mc + 1) * MCW]
                                .rearrange("(g p) m -> p g m", p=P))
                        wview = wo_sb[:].rearrange("p (g m) -> p g m", g=16)
                        for lt in range(2):
                            po = psC.tile([P, MCW], F32, tag="po", name="po")
                            for u in range(8):
                                nc.tensor.matmul(
                                    po[:],
                                    oview[:, 2 * u:2 * u + 2,
                                          lt * P:(lt + 1) * P],
                                    wview[:, 2 * u:2 * u + 2, :],
                                    start=(u == 0), stop=(u == 7),
                                    perf_mode=DRM)
                            dst = x_sb[lt][:, mc * MCW:(mc + 1) * MCW]
                            if ch == 0:
                                nc.scalar.activation(dst, po[:], COPY,
                                                     scale=SO)
                            else:
                                nc.vector.scalar_tensor_tensor(
                                    out=dst, in0=po[:], scalar=SO, in1=dst,
                                    op0=mybir.AluOpType.mult,
                                    op1=mybir.AluOpType.add)

                for lt in range(2):
                    hb_sb = pcs.tile([P, H], F32, tag="hb", name="hb_sb")
                    nc.sync.dma_start(out=hb_sb[:],
                                      in_=hb_d[lt * P:(lt + 1) * P, :])
                    x = x_sb[lt]
                    musum = pcs.tile([P, 1], F32, tag="musum", name="musum")
                    nc.vector.scalar_tensor_tensor(
                        out=x[:], in0=x[:], scalar=1.0, in1=hb_sb[:],
                        op0=mybir.AluOpType.mult, op1=mybir.AluOpType.add,
                        accum_out=musum[:])
                    mu = pcs.tile([P, 1], F32, tag="mu", name="mu")
                    nc.scalar.mul(mu[:], musum[:], 1.0 / H)
                    nc.vector.tensor_scalar(
                        out=x[:], in0=x[:], scalar1=mu[:], scalar2=None,
                        op0=mybir.AluOpType.subtract)
                    ssq = pcs.tile([P, 1], F32, tag="ssq", name="ssq")
                    nc.scalar.activation(
                        hb_sb[:], x[:], mybir.ActivationFunctionType.Square,
                        accum_out=ssq[:])
                    eps_sb = pcs.tile([P, 1], F32, tag="eps", name="eps")
                    nc.vector.memset(eps_sb[:], 1e-5)
                    std = pcs.tile([P, 1], F32, tag="std", name="std")
                    nc.scalar.activation(
                        std[:], ssq[:], mybir.ActivationFunctionType.Sqrt,
                        bias=eps_sb[:], scale=1.0 / H)
                    rstd = pcs.tile([P, 1], F32, tag="rstd", name="rstd")
                    nc.vector.reciprocal(rstd[:], std[:])
                    o_sb = pcs.tile([P, H], F32, tag="hb", name="o_sb")
                    nc.vector.scalar_tensor_tensor(
                        out=o_sb[:], in0=x[:], scalar=rstd[:], in1=g_sb[:],
                        op0=mybir.AluOpType.mult, op1=mybir.AluOpType.mult)
                    nc.vector.tensor_tensor(
                        out=o_sb[:], in0=o_sb[:], in1=bta_sb[:],
                        op=mybir.AluOpType.add)
                    nc.sync.dma_start(out=out_d[lt * P:(lt + 1) * P, :],
                                      in_=o_sb[:])

    nc.compile()
    return nc


def _prep_inputs(hidden_states, vision_features, attention_mask,
                 Wq, bq, Wk, bk, Wv, bv, Wo, bo, ln_g, ln_b):
    f = np.asarray
    hs = f(hidden_states, dtype=np.float32).reshape(L2, H)
    vf = f(vision_features, dtype=np.float32).reshape(L2, H)
    am = f(attention_mask)
    Wq, bq = f(Wq, dtype=np.float32), f(bq, dtype=np.float32)
    Wk, bk = f(Wk, dtype=np.float32), f(bk, dtype=np.float32)
    Wv, bv = f(Wv, dtype=np.float32), f(bv, dtype=np.float32)
    Wo, bo = f(Wo, dtype=np.float32), f(bo, dtype=np.float32)
    ln_g, ln_b = f(ln_g, dtype=np.float32), f(ln_b, dtype=np.float32)

    rs = 1.0 / np.sqrt(P)
    hidT = np.ascontiguousarray(hs.T).astype(FP8)
    visT = np.ascontiguousarray(vf.T).astype(FP8)
    # Wo host relayout: [ch, (i, ct, p), m] where source row = full ctx col
    # (i*4 + ch*2 + ct)*P + p, m = out col.  woT_full = Wo.T * WS.
    woT_full = (Wo.T * WS).astype(FP8)           # [H(ctx col), H(out col)]
    w5 = woT_full.reshape(NCORES, 2, 2, P, H)    # [i, ch, ct, p, m]
    woT = np.ascontiguousarray(
        w5.transpose(1, 0, 2, 3, 4).reshape(2, H // 2, H))
    mb = np.where(am != 0, 0.0, MSK).astype(np.float32)          # (B, LB)
    mskb = np.ascontiguousarray(
        mb.reshape(B, 8, P).transpose(2, 0, 1).reshape(P, B * 8))
    bo_eff = bo + Wo @ bv
    g_rep = np.ascontiguousarray(np.broadcast_to(ln_g, (P, H)))
    b_rep = np.ascontiguousarray(np.broadcast_to(ln_b, (P, H)))

    in_maps = []
    for c in range(NCORES):
        sl = slice(c * CW, (c + 1) * CW)
        in_maps.append({
            "hidT": hidT,
            "visT": visT,
            "wqT": np.ascontiguousarray((Wq[sl] * WS).T).astype(FP8),
            "wkT": np.ascontiguousarray((Wk[sl] * WS).T).astype(FP8),
            "wvT": np.ascontiguousarray((Wv[sl] * WS).T).astype(FP8),
            "woT": woT,
            "bqT": np.ascontiguousarray((bq[sl] * rs).reshape(NHL, P).T),
            "bkT": np.ascontiguousarray(bk[sl].reshape(NHL, P).T),
            "mskb": mskb,
            "hb": np.ascontiguousarray(hs[c * TL:(c + 1) * TL] + bo_eff),
            "g": g_rep,
            "bta": b_rep,
        })
    return in_maps


def kernel(**inputs) -> np.ndarray:
    inputs.pop("_debug", False)
    if "main" not in _CACHE:
        _CACHE["main"] = _build(debug=False)
    nc = _CACHE["main"]
    in_maps = _prep_inputs(**inputs)
    res = run_bass_kernel_spmd(nc, in_maps, list(range(NCORES)))
    out = np.concatenate([res.results[c]["out"] for c in range(NCORES)], axis=0)
    return out.reshape(B, LB, H)
